# revision 29
# baseline (speedup 1.0000x reference)
"""Distributed attention kernel for TRN2 (8 NeuronCores).

Computes: softmax(sqrt(Dqk) * (x@Wq.T) @ (x@Wk.T).T) @ (x@Wv.T)
for x [8192, 1024], Wq/Wk/Wv [256, 1024], out [8192, 256].

Sharding: rows of x across 8 cores (sequence parallel). Weights replicated.
Each core projects its shard, AllGathers K^T (f32) + V (bf16) in ONE packed
collective, then runs flash-style attention over its 1024 Q rows.

Per-core dataflow:
  - W DMA'd first and PE-transposed to W^T; x streamed per 128-row tile,
    PE-transposed to x^T (f32r rounding on the PSUM->SBUF eviction,
    alternating DVE/ACT), k^T/v projections pipelined behind the transposes.
  - k^T [dqk, nsh] f32r and v [nsh, dv] bf16 packed into one DRAM buffer,
    ONE AllGather; q^T projection (scaled by sqrt(dqk)) overlaps the AG.
  - K^T [dqk, N] / V [N, dv] assembled in SBUF by per-rank DMAs so the main
    loop can start as regions land.
  - per 128-row i-tile: scores chunks in PSUM (f32r matmul, 2 LDW per chunk
    pair), chunk row-max on DVE, exp on ACT (PSUM -> SBUF bf16, chunk-max
    bias, row-sums via accum_out), deferred max-correction scaling on GpSimd,
    P^T via DMA xbar transpose, PV matmul bf16, normalize, DMA out.
"""

import numpy as np

import concourse.bacc as bacc
import concourse.mybir as mybir
import concourse.tile as tile
from concourse.bass_utils import run_bass_kernel_spmd
from concourse.masks import make_identity

F32 = mybir.dt.float32
F32R = mybir.dt.float32r
BF16 = mybir.dt.bfloat16

N_CORES = 8
N, D, DQK, DV = 8192, 1024, 256, 256
P = 128
CHUNK = 1024  # scores chunk width (2 PSUM banks)


def build(n=N, d=D, dqk=DQK, dv=DV, ncores=N_CORES):
    nsh = n // ncores
    IT = nsh // P       # i-tiles per core
    KT = d // P         # contract tiles for projections
    CT = dqk // P       # dqk tiles
    JT = n // P         # j tiles for PV
    NCH = n // CHUNK    # score chunks per row
    NN = CHUNK // 512   # 512-wide matmuls per chunk
    scale = float(np.sqrt(dqk))

    # packed AG buffer: kT as [dqk, nsh] f32r (= [dqk, 2*nsh] bf16 rows),
    # v as [nsh*dv] bf16 flattened into rows of 2*nsh bf16.
    VROWS = (nsh * dv) // (2 * nsh)   # = dv // 2
    PACK_ROWS = dqk + VROWS

    nc = bacc.Bacc(None, target_bir_lowering=False, num_devices=ncores)

    x_ext = nc.declare_dram_parameter("x", [nsh, d], F32, isOutput=False)
    wq_ext = nc.declare_dram_parameter("Wq", [dqk, d], F32, isOutput=False)
    wk_ext = nc.declare_dram_parameter("Wk", [dqk, d], F32, isOutput=False)
    wv_ext = nc.declare_dram_parameter("Wv", [dv, d], F32, isOutput=False)
    out_ext = nc.declare_dram_parameter("out", [nsh, dv], F32, isOutput=True)

    U16 = mybir.dt.uint16
    pack = nc.dram_tensor("pack", [PACK_ROWS, 2 * nsh], U16)
    pack_ag = nc.dram_tensor(
        "pack_ag", [ncores * PACK_ROWS, 2 * nsh], U16, addr_space="Shared"
    )

    groups = [list(range(ncores))]

    with tile.TileContext(nc) as tc:
        with tc.tile_pool(name="persist", bufs=1) as pp:
            qt_s = pp.tile([P, CT, nsh], F32R, tag="qt")
            ident = pp.tile([P, P], F32, tag="ident")
            make_identity(nc, ident[:])

            # ================= Phase A =================
            with (
                tc.tile_pool(name="phA", bufs=1) as pa,
                tc.tile_pool(name="phA_psum", bufs=1, space="PSUM") as paps,
            ):
                x_nat = pa.tile([P, IT, d], F32, tag="xnat")
                for it in range(IT):
                    nc.sync.dma_start(
                        x_nat[:, it, :],
                        x_ext.ap().rearrange("(it p) d -> p it d", p=P)[:, it, :],
                    )
                # W order: v, k needed first (gate the AllGather); q last
                w_nat = pa.tile([P, 3 * CT, d], F32, tag="wnat")
                for wi, w_ext in enumerate((wq_ext, wk_ext, wv_ext)):
                    nc.scalar.dma_start(
                        w_nat[:, wi * CT:(wi + 1) * CT, :],
                        w_ext.ap().rearrange("(ct p) d -> p ct d", p=P),
                    )

                xt_s = pa.tile([P, KT, nsh], F32R, tag="xt")
                wt_s = pa.tile([P, 3 * KT, dqk], F32R, tag="wt")
                kt_loc = pa.tile([P, CT, nsh], F32R, tag="ktloc")
                v_loc = pa.tile([P, IT, dv], BF16, tag="vloc")

                ei = 0  # eviction engine alternator

                def evict(dst, src):
                    nonlocal ei
                    ei += 1
                    if ei % 2:
                        nc.vector.tensor_copy(dst, src)
                    else:
                        nc.scalar.copy(dst, src)

                # x transposes first (tiles land early), per tile as DMA'd
                ich_w = min(512, nsh)
                for it in range(IT):
                    for kt in range(KT):
                        tp = paps.tile([P, P], F32, tag="tp", bufs=4)
                        nc.tensor.transpose(
                            tp[:], x_nat[:, it, kt * P:(kt + 1) * P], ident[:]
                        )
                        evict(xt_s[:, kt, it * P:(it + 1) * P], tp[:])
                # Wv, Wk transposes (W DMAs landed meanwhile); Wq later
                for wi in (2, 1):
                    for kt in range(KT):
                        for ct in range(CT):
                            tp = paps.tile([P, P], F32, tag="tp", bufs=4)
                            nc.tensor.transpose(
                                tp[:],
                                w_nat[:, wi * CT + ct, kt * P:(kt + 1) * P],
                                ident[:],
                            )
                            evict(wt_s[:, wi * KT + kt, ct * P:(ct + 1) * P], tp[:])
                # v projections
                for it in range(IT):
                    psv = paps.tile([P, dv], F32, tag="psv", bufs=2)
                    for kt in range(KT):
                        nc.tensor.matmul(
                            psv[:],
                            xt_s[:, kt, it * P:(it + 1) * P],
                            wt_s[:, 2 * KT + kt, :dqk],
                            start=(kt == 0),
                            stop=(kt == KT - 1),
                        )
                    nc.vector.tensor_copy(v_loc[:, it, :], psv[:])
                # k^T projections
                for ich in range(nsh // ich_w):
                    for ct in range(CT):
                        psk = paps.tile([P, ich_w], F32, tag="pqk", bufs=2)
                        for kt in range(KT):
                            nc.tensor.matmul(
                                psk[:],
                                wt_s[:, 1 * KT + kt, ct * P:(ct + 1) * P],
                                xt_s[:, kt, ich * ich_w:(ich + 1) * ich_w],
                                start=(kt == 0),
                                stop=(kt == KT - 1),
                            )
                        nc.vector.tensor_copy(
                            kt_loc[:, ct, ich * ich_w:(ich + 1) * ich_w],
                            psk[:],
                        )

                # stage packed shard: kT rows then v rows
                nc.sync.dma_start(
                    pack.ap()[:dqk, :].bitcast(F32R).rearrange(
                        "(ct p) i -> p ct i", p=P
                    ),
                    kt_loc[:],
                )
                # v bytes: natural row-major into pack rows
                vpl = (2 * nsh) // dv   # p-values packed per AG row
                vph = P // vpl          # AG rows per x-tile
                for it in range(IT):
                    nc.sync.dma_start(
                        pack.ap()[dqk + it * vph:dqk + (it + 1) * vph, :]
                        .bitcast(BF16).rearrange("ph (pl c) -> ph pl c", pl=vpl),
                        v_loc[:, it, :],
                    )

                # Wq transposes + q^T projection (overlap the AllGather)
                for kt in range(KT):
                    for ct in range(CT):
                        tp = paps.tile([P, P], F32, tag="tp", bufs=4)
                        nc.tensor.transpose(
                            tp[:],
                            w_nat[:, 0 * CT + ct, kt * P:(kt + 1) * P],
                            ident[:],
                        )
                        evict(wt_s[:, 0 * KT + kt, ct * P:(ct + 1) * P], tp[:])
                n_ich = nsh // ich_w
                qt_src = []
                for ct in range(CT):
                    for ich in range(n_ich):
                        psq = paps.tile([P, ich_w], F32, tag="pqk", bufs=2)
                        for kt in range(KT):
                            nc.tensor.matmul(
                                psq[:],
                                wt_s[:, 0 * KT + kt, ct * P:(ct + 1) * P],
                                xt_s[:, kt, ich * ich_w:(ich + 1) * ich_w],
                                start=(kt == 0),
                                stop=(kt == KT - 1),
                            )
                        nc.vector.tensor_scalar_mul(
                            qt_s[:, ct, ich * ich_w:(ich + 1) * ich_w],
                            psq[:],
                            scale,
                        )

            # ================= one packed AllGather =================
            phb_cm = tc.tile_pool(name="phB", bufs=1)
            phb = phb_cm.__enter__()
            kt_full = phb.tile([P, CT, n], F32R, tag="ktf", name="kt_full")
            v_s = phb.tile([P, JT, dv], BF16, tag="vs", name="v_s")

            nc.gpsimd.collective_compute(
                "AllGather",
                mybir.AluOpType.bypass,
                replica_groups=groups,
                ins=[pack.ap().opt()],
                outs=[pack_ag.ap().opt()],
            )


            def assemble_rank(r):
                for ct in range(CT):
                    nc.sync.dma_start(
                        kt_full[:, ct, r * nsh:(r + 1) * nsh],
                        pack_ag.ap()[
                            r * PACK_ROWS + ct * P:
                            r * PACK_ROWS + (ct + 1) * P, :
                        ].bitcast(F32R),
                    )
                for it in range(IT):
                    base = r * PACK_ROWS + dqk + it * vph
                    nc.sync.dma_start(
                        v_s[:, r * IT + it, :],
                        pack_ag.ap()[base:base + vph, :].bitcast(BF16).rearrange(
                            "ph (pl c) -> ph pl c", pl=vpl
                        ),
                    )

            # ================= Main attention loop =================
            # Streaming flash-attention: each 1024-wide chunk flows
            # MM -> row-max -> exp -> P^T transpose -> PV partial, with a
            # running-max rescale of the accumulator (no per-i-tile barrier).
            with (
                tc.tile_pool(name="mainA", bufs=8) as ma,
                tc.tile_pool(name="chunkp", bufs=6) as cp,
                tc.tile_pool(name="scores_psum", bufs=3, space="PSUM") as sps,
                tc.tile_pool(name="out_psum", bufs=2, space="PSUM") as ops,
            ):
                JPC = CHUNK // P  # j-tiles per chunk

                # flatten (i-tile, chunk) into one stream; PV lags scores by
                # one chunk so the PE never stalls on the exp->transpose tail
                NTOT = IT * NCH
                stats = []
                for it in range(IT):
                    st = {
                        "mneg": ma.tile([P, NCH], F32, tag="mneg", name="mneg"),
                        "sig": ma.tile([P, NCH], F32, tag="sig", name="sig"),
                        "nmr": ma.tile([P, NCH], F32, tag="nmr", name="nmr"),
                        "alpha": ma.tile([P, NCH], F32, tag="alpha", name="alpha"),
                        "asig": ma.tile([P, NCH], F32, tag="asig", name="asig"),
                        "rs": ma.tile([P, 1], F32, tag="rs", name="rs"),
                        "rinv": ma.tile([P, 1], F32, tag="rinv", name="rinv"),
                        "gam": ma.tile([P, NCH], F32, tag="gam", name="gam"),
                        "bet": ma.tile([P, NCH], F32, tag="bet", name="bet"),
                        "acc": ma.tile([P, dv], F32, tag="acc", name="acc"),
                    }
                    stats.append(st)

                pend = None  # (it, ch, pt_c, po-alloc deferred)

                def do_scores(k):
                    ch, it = divmod(k, IT)
                    st = stats[it]
                    ps = sps.tile([P, CHUNK], F32, tag="s", name="ps")
                    for ct in range(CT):
                        for nn in range(NN):
                            nc.tensor.matmul(
                                ps[:, nn * 512:(nn + 1) * 512],
                                qt_s[:, ct, it * P:(it + 1) * P],
                                kt_full[
                                    :, ct,
                                    ch * CHUNK + nn * 512:
                                    ch * CHUNK + (nn + 1) * 512,
                                ],
                                start=(ct == 0),
                                stop=(ct == CT - 1),
                                skip_group_check=True,
                            )
                    nc.vector.reduce_max(
                        st["mneg"][:, ch:ch + 1], ps[:],
                        axis=mybir.AxisListType.X, negate=True,
                    )
                    if ch > 0:
                        nc.vector.tensor_tensor(
                            st["nmr"][:, ch:ch + 1], st["nmr"][:, ch - 1:ch],
                            st["mneg"][:, ch:ch + 1], op=mybir.AluOpType.min,
                        )
                    else:
                        nc.vector.tensor_copy(st["nmr"][:, :1], st["mneg"][:, :1])
                    p_c = cp.tile([P, CHUNK], BF16, tag="p", name="p_c")
                    nc.scalar.activation(
                        p_c[:], ps[:],
                        mybir.ActivationFunctionType.Exp,
                        bias=st["mneg"][:, ch:ch + 1],
                        scale=1.0,
                        accum_out=st["sig"][:, ch:ch + 1],
                    )
                    pt_c = cp.tile([P, JPC, P], BF16, tag="pt", name="pt_c")
                    nc.scalar.dma_start_transpose(pt_c[:], p_c[:])
                    return pt_c

                def do_pv(k, pt_c):
                    ch, it = divmod(k, IT)
                    st = stats[it]
                    po = ops.tile([P, dv], F32, tag="po", name="po")
                    for j2 in range(JPC):
                        nc.tensor.matmul(
                            po[:], pt_c[:, j2, :],
                            v_s[:, ch * JPC + j2, :],
                            start=(j2 == 0), stop=(j2 == JPC - 1),
                        )
                    if ch == 0:
                        nc.vector.tensor_copy(st["acc"][:], po[:])
                    else:
                        nc.scalar.activation(
                            st["gam"][:, ch:ch + 1], st["nmr"][:, ch - 1:ch],
                            mybir.ActivationFunctionType.Exp,
                            bias=st["nmr"][:, ch:ch + 1], scale=-1.0,
                        )
                        nc.scalar.activation(
                            st["bet"][:, ch:ch + 1], st["mneg"][:, ch:ch + 1],
                            mybir.ActivationFunctionType.Exp,
                            bias=st["nmr"][:, ch:ch + 1], scale=-1.0,
                        )
                        nc.vector.tensor_scalar_mul(
                            st["acc"][:], st["acc"][:], st["gam"][:, ch:ch + 1]
                        )
                        nc.vector.scalar_tensor_tensor(
                            st["acc"][:], po[:], st["bet"][:, ch:ch + 1],
                            st["acc"][:],
                            op0=mybir.AluOpType.mult,
                            op1=mybir.AluOpType.add,
                        )
                    if ch == NCH - 1:
                        nc.scalar.activation(
                            st["alpha"][:], st["mneg"][:],
                            mybir.ActivationFunctionType.Exp,
                            bias=st["nmr"][:, NCH - 1:NCH], scale=-1.0,
                        )
                        nc.vector.tensor_mul(st["asig"][:], st["alpha"][:], st["sig"][:])
                        nc.vector.reduce_sum(
                            st["rs"][:], st["asig"][:], axis=mybir.AxisListType.X
                        )
                        nc.vector.reciprocal(st["rinv"][:], st["rs"][:])
                        nc.vector.tensor_scalar_mul(
                            st["acc"][:], st["acc"][:], st["rinv"][:]
                        )
                        nc.sync.dma_start(
                            out_ext.ap().rearrange("(it p) c -> p it c", p=P)[
                                :, it, :
                            ],
                            st["acc"][:],
                        )

                order = [g * IT + i2 for g in range(NCH) for i2 in range(IT)]
                for r in range(ncores):
                    assemble_rank(r)
                for k in order:
                    pt_now = do_scores(k)
                    if pend is not None:
                        do_pv(pend[0], pend[1])
                    pend = (k, pt_now)
                do_pv(pend[0], pend[1])

            phb_cm.__exit__(None, None, None)

    nc.finalize()
    return nc


_NC_CACHE = {}


def _get_nc(key):
    if key not in _NC_CACHE:
        n, d, dqk, dv, ncores = key
        _NC_CACHE[key] = build(n=n, d=d, dqk=dqk, dv=dv, ncores=ncores)
    return _NC_CACHE[key]


def run(x, Wq, Wk, Wv, trace=False):
    n, d = x.shape
    dqk = Wq.shape[0]
    dv = Wv.shape[0]
    ncores = N_CORES
    nsh = n // ncores
    nc = _get_nc((n, d, dqk, dv, ncores))

    x = np.ascontiguousarray(x, dtype=np.float32)
    Wq = np.ascontiguousarray(Wq, dtype=np.float32)
    Wk = np.ascontiguousarray(Wk, dtype=np.float32)
    Wv = np.ascontiguousarray(Wv, dtype=np.float32)

    in_maps = [
        {"x": x[r * nsh:(r + 1) * nsh], "Wq": Wq, "Wk": Wk, "Wv": Wv}
        for r in range(ncores)
    ]
    res = run_bass_kernel_spmd(
        nc, in_maps, core_ids=list(range(ncores)), trace=trace
    )
    out = np.concatenate([res.results[r]["out"] for r in range(ncores)], axis=0)
    return out, res


def kernel(x, Wq, Wk, Wv):
    out, _ = run(x, Wq, Wk, Wv)
    return out


# revision 32
# speedup vs baseline: 1.3050x; 1.3050x over previous
"""Distributed attention kernel for TRN2 (8 NeuronCores).

Computes: softmax(sqrt(Dqk) * (x@Wq.T) @ (x@Wk.T).T) @ (x@Wv.T)
for x [8192, 1024], Wq/Wk/Wv [256, 1024], out [8192, 256].

Sharding: rows of x across 8 cores (sequence parallel). Weights replicated.
Each core projects its shard, AllGathers K^T (f32) + V (bf16) in ONE packed
collective, then runs flash-style attention over its 1024 Q rows.

Per-core dataflow:
  - W DMA'd first and PE-transposed to W^T; x streamed per 128-row tile,
    PE-transposed to x^T (f32r rounding on the PSUM->SBUF eviction,
    alternating DVE/ACT), k^T/v projections pipelined behind the transposes.
  - k^T [dqk, nsh] f32r and v [nsh, dv] bf16 packed into one DRAM buffer,
    ONE AllGather; q^T projection (scaled by sqrt(dqk)) overlaps the AG.
  - K^T [dqk, N] / V [N, dv] assembled in SBUF by per-rank DMAs so the main
    loop can start as regions land.
  - per 128-row i-tile: scores chunks in PSUM (f32r matmul, 2 LDW per chunk
    pair), chunk row-max on DVE, exp on ACT (PSUM -> SBUF bf16, chunk-max
    bias, row-sums via accum_out), deferred max-correction scaling on GpSimd,
    P^T via DMA xbar transpose, PV matmul bf16, normalize, DMA out.
"""

import numpy as np

import concourse.bacc as bacc
import concourse.mybir as mybir
import concourse.tile as tile
from concourse.bass_utils import run_bass_kernel_spmd
from concourse.masks import make_identity

F32 = mybir.dt.float32
F32R = mybir.dt.float32r
BF16 = mybir.dt.bfloat16

N_CORES = 8
N, D, DQK, DV = 8192, 1024, 256, 256
P = 128
CHUNK = 1024  # scores chunk width (2 PSUM banks)


def build(n=N, d=D, dqk=DQK, dv=DV, ncores=N_CORES):
    nsh = n // ncores
    IT = nsh // P       # i-tiles per core
    KT = d // P         # contract tiles for projections
    CT = dqk // P       # dqk tiles
    JT = n // P         # j tiles for PV
    NCH = n // CHUNK    # score chunks per row
    NN = CHUNK // 512   # 512-wide matmuls per chunk
    scale = float(np.sqrt(dqk))

    # packed AG buffer: kT as [dqk, nsh] f32r (= [dqk, 2*nsh] bf16 rows),
    # v as [nsh*dv] bf16 flattened into rows of 2*nsh bf16.
    VROWS = (nsh * dv) // (2 * nsh)   # = dv // 2
    PACK_ROWS = dqk + VROWS

    nc = bacc.Bacc(None, target_bir_lowering=False, num_devices=ncores)

    x_ext = nc.declare_dram_parameter("x", [nsh, d], F32, isOutput=False)
    wq_ext = nc.declare_dram_parameter("Wq", [dqk, d], F32, isOutput=False)
    wk_ext = nc.declare_dram_parameter("Wk", [dqk, d], F32, isOutput=False)
    wv_ext = nc.declare_dram_parameter("Wv", [dv, d], F32, isOutput=False)
    out_ext = nc.declare_dram_parameter("out", [nsh, dv], F32, isOutput=True)

    U16 = mybir.dt.uint16

    groups = [list(range(ncores))]

    with tile.TileContext(nc) as tc:
        with (
            tc.tile_pool(name="persist", bufs=1) as pp,
            tc.tile_pool(name="dramp", bufs=1, space="DRAM") as dp,
        ):
            pack = dp.tile([PACK_ROWS, 2 * nsh], U16, name="pack")
            pack_ag = dp.tile(
                [ncores * PACK_ROWS, 2 * nsh], U16, addr_space="Shared",
                name="pack_ag",
            )
            qt_s = pp.tile([P, CT, nsh], F32R, tag="qt")
            ident = pp.tile([P, P], F32, tag="ident")
            make_identity(nc, ident[:])

            # ================= Phase A =================
            with (
                tc.tile_pool(name="phA", bufs=1) as pa,
                tc.tile_pool(name="phA_psum", bufs=1, space="PSUM") as paps,
            ):
                x_nat = pa.tile([P, IT, d], F32, tag="xnat")
                for it in range(IT):
                    nc.sync.dma_start(
                        x_nat[:, it, :],
                        x_ext.ap().rearrange("(it p) d -> p it d", p=P)[:, it, :],
                    )
                # W order: v, k needed first (gate the AllGather); q last
                w_nat = pa.tile([P, 3 * CT, d], F32, tag="wnat")
                for wi, w_ext in enumerate((wq_ext, wk_ext, wv_ext)):
                    nc.scalar.dma_start(
                        w_nat[:, wi * CT:(wi + 1) * CT, :],
                        w_ext.ap().rearrange("(ct p) d -> p ct d", p=P),
                    )

                xt_s = pa.tile([P, KT, nsh], F32R, tag="xt")
                wt_s = pa.tile([P, 3 * KT, dqk], F32R, tag="wt")
                kt_loc = pa.tile([P, CT, nsh], F32R, tag="ktloc")
                v_loc = pa.tile([P, IT, dv], BF16, tag="vloc")

                ei = 0  # eviction engine alternator

                def evict(dst, src):
                    nonlocal ei
                    ei += 1
                    if ei % 2:
                        nc.vector.tensor_copy(dst, src)
                    else:
                        nc.scalar.copy(dst, src)

                # x transposes first (tiles land early), per tile as DMA'd
                ich_w = min(512, nsh)
                for it in range(IT):
                    for kt in range(KT):
                        tp = paps.tile([P, P], F32, tag="tp", bufs=4)
                        nc.tensor.transpose(
                            tp[:], x_nat[:, it, kt * P:(kt + 1) * P], ident[:]
                        )
                        evict(xt_s[:, kt, it * P:(it + 1) * P], tp[:])
                # Wv, Wk transposes (W DMAs landed meanwhile); Wq later
                for wi in (2, 1):
                    for kt in range(KT):
                        for ct in range(CT):
                            tp = paps.tile([P, P], F32, tag="tp", bufs=4)
                            nc.tensor.transpose(
                                tp[:],
                                w_nat[:, wi * CT + ct, kt * P:(kt + 1) * P],
                                ident[:],
                            )
                            evict(wt_s[:, wi * KT + kt, ct * P:(ct + 1) * P], tp[:])
                # v projections
                for it in range(IT):
                    psv = paps.tile([P, dv], F32, tag="psv", bufs=2)
                    for kt in range(KT):
                        nc.tensor.matmul(
                            psv[:],
                            xt_s[:, kt, it * P:(it + 1) * P],
                            wt_s[:, 2 * KT + kt, :dqk],
                            start=(kt == 0),
                            stop=(kt == KT - 1),
                        )
                    nc.vector.tensor_copy(v_loc[:, it, :], psv[:])
                # k^T projections
                for ich in range(nsh // ich_w):
                    for ct in range(CT):
                        psk = paps.tile([P, ich_w], F32, tag="pqk", bufs=2)
                        for kt in range(KT):
                            nc.tensor.matmul(
                                psk[:],
                                wt_s[:, 1 * KT + kt, ct * P:(ct + 1) * P],
                                xt_s[:, kt, ich * ich_w:(ich + 1) * ich_w],
                                start=(kt == 0),
                                stop=(kt == KT - 1),
                            )
                        nc.vector.tensor_copy(
                            kt_loc[:, ct, ich * ich_w:(ich + 1) * ich_w],
                            psk[:],
                        )

                # stage packed shard: kT rows then v rows
                nc.sync.dma_start(
                    pack[:][:dqk, :].bitcast(F32R).rearrange(
                        "(ct p) i -> p ct i", p=P
                    ),
                    kt_loc[:],
                )
                # v bytes: natural row-major into pack rows
                vpl = (2 * nsh) // dv   # p-values packed per AG row
                vph = P // vpl          # AG rows per x-tile
                for it in range(IT):
                    nc.sync.dma_start(
                        pack[:][dqk + it * vph:dqk + (it + 1) * vph, :]
                        .bitcast(BF16).rearrange("ph (pl c) -> ph pl c", pl=vpl),
                        v_loc[:, it, :],
                    )

                # Wq transposes + q^T projection (overlap the AllGather)
                for kt in range(KT):
                    for ct in range(CT):
                        tp = paps.tile([P, P], F32, tag="tp", bufs=4)
                        nc.tensor.transpose(
                            tp[:],
                            w_nat[:, 0 * CT + ct, kt * P:(kt + 1) * P],
                            ident[:],
                        )
                        evict(wt_s[:, 0 * KT + kt, ct * P:(ct + 1) * P], tp[:])
                n_ich = nsh // ich_w
                qt_src = []
                for ct in range(CT):
                    for ich in range(n_ich):
                        psq = paps.tile([P, ich_w], F32, tag="pqk", bufs=2)
                        for kt in range(KT):
                            nc.tensor.matmul(
                                psq[:],
                                wt_s[:, 0 * KT + kt, ct * P:(ct + 1) * P],
                                xt_s[:, kt, ich * ich_w:(ich + 1) * ich_w],
                                start=(kt == 0),
                                stop=(kt == KT - 1),
                            )
                        nc.vector.tensor_scalar_mul(
                            qt_s[:, ct, ich * ich_w:(ich + 1) * ich_w],
                            psq[:],
                            scale,
                        )

            # ================= one packed AllGather =================
            phb_cm = tc.tile_pool(name="phB", bufs=1)
            phb = phb_cm.__enter__()
            kt_full = phb.tile([P, CT, n], F32R, tag="ktf", name="kt_full")
            v_s = phb.tile([P, JT, dv], BF16, tag="vs", name="v_s")

            nc.gpsimd.collective_compute(
                "AllGather",
                mybir.AluOpType.bypass,
                replica_groups=groups,
                ins=[pack[:].opt()],
                outs=[pack_ag[:].opt()],
            )


            def assemble_rank(r):
                for ct in range(CT):
                    eng = nc.sync if ct % 2 == 0 else nc.scalar
                    eng.dma_start(
                        kt_full[:, ct, r * nsh:(r + 1) * nsh],
                        pack_ag[:][
                            r * PACK_ROWS + ct * P:
                            r * PACK_ROWS + (ct + 1) * P, :
                        ].bitcast(F32R),
                    )
                for it in range(IT):
                    base = r * PACK_ROWS + dqk + it * vph
                    eng = nc.sync if it % 2 == 0 else nc.scalar
                    eng.dma_start(
                        v_s[:, r * IT + it, :],
                        pack_ag[:][base:base + vph, :].bitcast(BF16).rearrange(
                            "ph (pl c) -> ph pl c", pl=vpl
                        ),
                    )

            # ================= Main attention loop =================
            # Streaming flash-attention: each 1024-wide chunk flows
            # MM -> row-max -> exp -> P^T transpose -> PV partial, with a
            # running-max rescale of the accumulator (no per-i-tile barrier).
            with (
                tc.tile_pool(name="mainA", bufs=8) as ma,
                tc.tile_pool(name="chunkp", bufs=8) as cp,
                tc.tile_pool(name="scores_psum", bufs=3, space="PSUM") as sps,
                tc.tile_pool(name="out_psum", bufs=2, space="PSUM") as ops,
            ):
                JPC = CHUNK // P  # j-tiles per chunk

                # flatten (i-tile, chunk) into one stream; PV lags scores by
                # one chunk so the PE never stalls on the exp->transpose tail
                NTOT = IT * NCH
                stats = []
                for it in range(IT):
                    st = {
                        "mneg": ma.tile([P, NCH], F32, tag="mneg", name="mneg"),
                        "sig": ma.tile([P, NCH], F32, tag="sig", name="sig"),
                        "nmr": ma.tile([P, NCH], F32, tag="nmr", name="nmr"),
                        "alpha": ma.tile([P, NCH], F32, tag="alpha", name="alpha"),
                        "asig": ma.tile([P, NCH], F32, tag="asig", name="asig"),
                        "rs": ma.tile([P, 1], F32, tag="rs", name="rs"),
                        "rinv": ma.tile([P, 1], F32, tag="rinv", name="rinv"),
                        "gam": ma.tile([P, NCH], F32, tag="gam", name="gam"),
                        "bet": ma.tile([P, NCH], F32, tag="bet", name="bet"),
                        "acc": ma.tile([P, dv], F32, tag="acc", name="acc"),
                    }
                    stats.append(st)

                pend = None  # (it, ch, pt_c, po-alloc deferred)

                def do_scores(k):
                    ch, it = divmod(k, IT)
                    st = stats[it]
                    ps = sps.tile([P, CHUNK], F32, tag="s", name="ps")
                    for ct in range(CT):
                        for nn in range(NN):
                            nc.tensor.matmul(
                                ps[:, nn * 512:(nn + 1) * 512],
                                qt_s[:, ct, it * P:(it + 1) * P],
                                kt_full[
                                    :, ct,
                                    ch * CHUNK + nn * 512:
                                    ch * CHUNK + (nn + 1) * 512,
                                ],
                                start=(ct == 0),
                                stop=(ct == CT - 1),
                                skip_group_check=True,
                            )
                    nc.vector.reduce_max(
                        st["mneg"][:, ch:ch + 1], ps[:],
                        axis=mybir.AxisListType.X, negate=True,
                    )
                    if ch > 0:
                        nc.vector.tensor_tensor(
                            st["nmr"][:, ch:ch + 1], st["nmr"][:, ch - 1:ch],
                            st["mneg"][:, ch:ch + 1], op=mybir.AluOpType.min,
                        )
                    else:
                        nc.vector.tensor_copy(st["nmr"][:, :1], st["mneg"][:, :1])
                    p_c = cp.tile([P, CHUNK], BF16, tag="p", name="p_c")
                    nc.scalar.activation(
                        p_c[:], ps[:],
                        mybir.ActivationFunctionType.Exp,
                        bias=st["mneg"][:, ch:ch + 1],
                        scale=1.0,
                        accum_out=st["sig"][:, ch:ch + 1],
                    )
                    pt_c = cp.tile([P, JPC, P], BF16, tag="pt", name="pt_c")
                    nc.sync.dma_start_transpose(pt_c[:], p_c[:])
                    return pt_c

                def do_pv(k, pt_c):
                    ch, it = divmod(k, IT)
                    st = stats[it]
                    po = ops.tile([P, dv], F32, tag="po", name="po")
                    for j2 in range(JPC):
                        nc.tensor.matmul(
                            po[:], pt_c[:, j2, :],
                            v_s[:, ch * JPC + j2, :],
                            start=(j2 == 0), stop=(j2 == JPC - 1),
                        )
                    if ch == 0:
                        nc.vector.tensor_copy(st["acc"][:], po[:])
                    else:
                        nc.scalar.activation(
                            st["gam"][:, ch:ch + 1], st["nmr"][:, ch - 1:ch],
                            mybir.ActivationFunctionType.Exp,
                            bias=st["nmr"][:, ch:ch + 1], scale=-1.0,
                        )
                        nc.scalar.activation(
                            st["bet"][:, ch:ch + 1], st["mneg"][:, ch:ch + 1],
                            mybir.ActivationFunctionType.Exp,
                            bias=st["nmr"][:, ch:ch + 1], scale=-1.0,
                        )
                        nc.vector.tensor_scalar_mul(
                            st["acc"][:], st["acc"][:], st["gam"][:, ch:ch + 1]
                        )
                        nc.vector.scalar_tensor_tensor(
                            st["acc"][:], po[:], st["bet"][:, ch:ch + 1],
                            st["acc"][:],
                            op0=mybir.AluOpType.mult,
                            op1=mybir.AluOpType.add,
                        )
                    if ch == NCH - 1:
                        nc.scalar.activation(
                            st["alpha"][:], st["mneg"][:],
                            mybir.ActivationFunctionType.Exp,
                            bias=st["nmr"][:, NCH - 1:NCH], scale=-1.0,
                        )
                        nc.vector.tensor_mul(st["asig"][:], st["alpha"][:], st["sig"][:])
                        nc.vector.reduce_sum(
                            st["rs"][:], st["asig"][:], axis=mybir.AxisListType.X
                        )
                        nc.vector.reciprocal(st["rinv"][:], st["rs"][:])
                        nc.vector.tensor_scalar_mul(
                            st["acc"][:], st["acc"][:], st["rinv"][:]
                        )
                        nc.sync.dma_start(
                            out_ext.ap().rearrange("(it p) c -> p it c", p=P)[
                                :, it, :
                            ],
                            st["acc"][:],
                        )

                order = [g * IT + i2 for g in range(NCH) for i2 in range(IT)]
                for r in range(ncores):
                    assemble_rank(r)
                for k in order:
                    pt_now = do_scores(k)
                    if pend is not None:
                        do_pv(pend[0], pend[1])
                    pend = (k, pt_now)
                do_pv(pend[0], pend[1])

            phb_cm.__exit__(None, None, None)

    nc.finalize()
    return nc


_NC_CACHE = {}


def _get_nc(key):
    if key not in _NC_CACHE:
        n, d, dqk, dv, ncores = key
        _NC_CACHE[key] = build(n=n, d=d, dqk=dqk, dv=dv, ncores=ncores)
    return _NC_CACHE[key]


def run(x, Wq, Wk, Wv, trace=False):
    n, d = x.shape
    dqk = Wq.shape[0]
    dv = Wv.shape[0]
    ncores = N_CORES
    nsh = n // ncores
    nc = _get_nc((n, d, dqk, dv, ncores))

    x = np.ascontiguousarray(x, dtype=np.float32)
    Wq = np.ascontiguousarray(Wq, dtype=np.float32)
    Wk = np.ascontiguousarray(Wk, dtype=np.float32)
    Wv = np.ascontiguousarray(Wv, dtype=np.float32)

    in_maps = [
        {"x": x[r * nsh:(r + 1) * nsh], "Wq": Wq, "Wk": Wk, "Wv": Wv}
        for r in range(ncores)
    ]
    res = run_bass_kernel_spmd(
        nc, in_maps, core_ids=list(range(ncores)), trace=trace
    )
    out = np.concatenate([res.results[r]["out"] for r in range(ncores)], axis=0)
    return out, res


def kernel(x, Wq, Wk, Wv):
    out, _ = run(x, Wq, Wk, Wv)
    return out


# revision 33
# speedup vs baseline: 1.3282x; 1.0178x over previous
"""Distributed attention kernel for TRN2 (8 NeuronCores).

Computes: softmax(sqrt(Dqk) * (x@Wq.T) @ (x@Wk.T).T) @ (x@Wv.T)
for x [8192, 1024], Wq/Wk/Wv [256, 1024], out [8192, 256].

Sharding: rows of x across 8 cores (sequence parallel). Weights replicated.
Each core projects its shard, AllGathers K^T (f32) + V (bf16) in ONE packed
collective, then runs flash-style attention over its 1024 Q rows.

Per-core dataflow:
  - W DMA'd first and PE-transposed to W^T; x streamed per 128-row tile,
    PE-transposed to x^T (f32r rounding on the PSUM->SBUF eviction,
    alternating DVE/ACT), k^T/v projections pipelined behind the transposes.
  - k^T [dqk, nsh] f32r and v [nsh, dv] bf16 packed into one DRAM buffer,
    ONE AllGather; q^T projection (scaled by sqrt(dqk)) overlaps the AG.
  - K^T [dqk, N] / V [N, dv] assembled in SBUF by per-rank DMAs so the main
    loop can start as regions land.
  - per 128-row i-tile: scores chunks in PSUM (f32r matmul, 2 LDW per chunk
    pair), chunk row-max on DVE, exp on ACT (PSUM -> SBUF bf16, chunk-max
    bias, row-sums via accum_out), deferred max-correction scaling on GpSimd,
    P^T via DMA xbar transpose, PV matmul bf16, normalize, DMA out.
"""

import numpy as np

import concourse.bacc as bacc
import concourse.mybir as mybir
import concourse.tile as tile
from concourse.bass_utils import run_bass_kernel_spmd
from concourse.masks import make_identity

F32 = mybir.dt.float32
F32R = mybir.dt.float32r
BF16 = mybir.dt.bfloat16

N_CORES = 8
N, D, DQK, DV = 8192, 1024, 256, 256
P = 128
CHUNK = 1024  # scores chunk width (2 PSUM banks)


def build(n=N, d=D, dqk=DQK, dv=DV, ncores=N_CORES):
    nsh = n // ncores
    IT = nsh // P       # i-tiles per core
    KT = d // P         # contract tiles for projections
    CT = dqk // P       # dqk tiles
    JT = n // P         # j tiles for PV
    NCH = n // CHUNK    # score chunks per row
    NN = CHUNK // 512   # 512-wide matmuls per chunk
    scale = float(np.sqrt(dqk))

    # packed AG buffer: kT as [dqk, nsh] f32r (= [dqk, 2*nsh] bf16 rows),
    # v as [nsh*dv] bf16 flattened into rows of 2*nsh bf16.
    VROWS = (nsh * dv) // (2 * nsh)   # = dv // 2
    PACK_ROWS = dqk + VROWS

    nc = bacc.Bacc(None, target_bir_lowering=False, num_devices=ncores)

    x_ext = nc.declare_dram_parameter("x", [nsh, d], F32, isOutput=False)
    wq_ext = nc.declare_dram_parameter("Wq", [dqk, d], F32, isOutput=False)
    wk_ext = nc.declare_dram_parameter("Wk", [dqk, d], F32, isOutput=False)
    wv_ext = nc.declare_dram_parameter("Wv", [dv, d], F32, isOutput=False)
    out_ext = nc.declare_dram_parameter("out", [nsh, dv], F32, isOutput=True)

    U16 = mybir.dt.uint16

    groups = [list(range(ncores))]

    with tile.TileContext(nc) as tc:
        with (
            tc.tile_pool(name="persist", bufs=1) as pp,
            tc.tile_pool(name="dramp", bufs=1, space="DRAM") as dp,
        ):
            pack = dp.tile([PACK_ROWS, 2 * nsh], U16, name="pack")
            pack_ag = dp.tile(
                [ncores * PACK_ROWS, 2 * nsh], U16, addr_space="Shared",
                name="pack_ag",
            )
            qt_s = pp.tile([P, CT, nsh], F32R, tag="qt")
            ident = pp.tile([P, P], F32, tag="ident")
            make_identity(nc, ident[:])

            # ================= Phase A =================
            with (
                tc.tile_pool(name="phA", bufs=1) as pa,
                tc.tile_pool(name="phA_psum", bufs=1, space="PSUM") as paps,
            ):
                x_nat = pa.tile([P, IT, d], F32, tag="xnat")
                for it in range(IT):
                    nc.sync.dma_start(
                        x_nat[:, it, :],
                        x_ext.ap().rearrange("(it p) d -> p it d", p=P)[:, it, :],
                    )
                # W order: v, k needed first (gate the AllGather); q last
                w_nat = pa.tile([P, 3 * CT, d], F32, tag="wnat")
                for wi, w_ext in enumerate((wq_ext, wk_ext, wv_ext)):
                    nc.scalar.dma_start(
                        w_nat[:, wi * CT:(wi + 1) * CT, :],
                        w_ext.ap().rearrange("(ct p) d -> p ct d", p=P),
                    )

                xt_s = pa.tile([P, KT, nsh], F32R, tag="xt")
                wt_s = pa.tile([P, 3 * KT, dqk], F32R, tag="wt")
                kt_loc = pa.tile([P, CT, nsh], F32R, tag="ktloc")
                v_loc = pa.tile([P, IT, dv], BF16, tag="vloc")

                ei = 0  # eviction engine alternator

                def evict(dst, src):
                    nonlocal ei
                    ei += 1
                    if ei % 2:
                        nc.vector.tensor_copy(dst, src)
                    else:
                        nc.scalar.copy(dst, src)

                # x transposes first (tiles land early), per tile as DMA'd
                ich_w = min(512, nsh)
                for it in range(IT):
                    for kt in range(KT):
                        tp = paps.tile([P, P], F32, tag="tp", bufs=4)
                        nc.tensor.transpose(
                            tp[:], x_nat[:, it, kt * P:(kt + 1) * P], ident[:]
                        )
                        evict(xt_s[:, kt, it * P:(it + 1) * P], tp[:])
                # Wv, Wk transposes (W DMAs landed meanwhile); Wq later
                for wi in (2, 1):
                    for kt in range(KT):
                        for ct in range(CT):
                            tp = paps.tile([P, P], F32, tag="tp", bufs=4)
                            nc.tensor.transpose(
                                tp[:],
                                w_nat[:, wi * CT + ct, kt * P:(kt + 1) * P],
                                ident[:],
                            )
                            evict(wt_s[:, wi * KT + kt, ct * P:(ct + 1) * P], tp[:])
                # v projections
                for it in range(IT):
                    psv = paps.tile([P, dv], F32, tag="psv", bufs=2)
                    for kt in range(KT):
                        nc.tensor.matmul(
                            psv[:],
                            xt_s[:, kt, it * P:(it + 1) * P],
                            wt_s[:, 2 * KT + kt, :dqk],
                            start=(kt == 0),
                            stop=(kt == KT - 1),
                        )
                    nc.vector.tensor_copy(v_loc[:, it, :], psv[:])
                # k^T projections
                for ich in range(nsh // ich_w):
                    for ct in range(CT):
                        psk = paps.tile([P, ich_w], F32, tag="pqk", bufs=2)
                        for kt in range(KT):
                            nc.tensor.matmul(
                                psk[:],
                                wt_s[:, 1 * KT + kt, ct * P:(ct + 1) * P],
                                xt_s[:, kt, ich * ich_w:(ich + 1) * ich_w],
                                start=(kt == 0),
                                stop=(kt == KT - 1),
                            )
                        nc.vector.tensor_copy(
                            kt_loc[:, ct, ich * ich_w:(ich + 1) * ich_w],
                            psk[:],
                        )

                # stage packed shard: kT rows then v rows
                nc.sync.dma_start(
                    pack[:][:dqk, :].bitcast(F32R).rearrange(
                        "(ct p) i -> p ct i", p=P
                    ),
                    kt_loc[:],
                )
                # v bytes: natural row-major into pack rows
                vpl = (2 * nsh) // dv   # p-values packed per AG row
                vph = P // vpl          # AG rows per x-tile
                for it in range(IT):
                    nc.sync.dma_start(
                        pack[:][dqk + it * vph:dqk + (it + 1) * vph, :]
                        .bitcast(BF16).rearrange("ph (pl c) -> ph pl c", pl=vpl),
                        v_loc[:, it, :],
                    )

                # Wq transposes + q^T projection (overlap the AllGather)
                for kt in range(KT):
                    for ct in range(CT):
                        tp = paps.tile([P, P], F32, tag="tp", bufs=4)
                        nc.tensor.transpose(
                            tp[:],
                            w_nat[:, 0 * CT + ct, kt * P:(kt + 1) * P],
                            ident[:],
                        )
                        evict(wt_s[:, 0 * KT + kt, ct * P:(ct + 1) * P], tp[:])
                n_ich = nsh // ich_w
                qt_src = []
                for ct in range(CT):
                    for ich in range(n_ich):
                        psq = paps.tile([P, ich_w], F32, tag="pqk", bufs=2)
                        for kt in range(KT):
                            nc.tensor.matmul(
                                psq[:],
                                wt_s[:, 0 * KT + kt, ct * P:(ct + 1) * P],
                                xt_s[:, kt, ich * ich_w:(ich + 1) * ich_w],
                                start=(kt == 0),
                                stop=(kt == KT - 1),
                            )
                        nc.vector.tensor_scalar_mul(
                            qt_s[:, ct, ich * ich_w:(ich + 1) * ich_w],
                            psq[:],
                            scale,
                        )

            # ================= one packed AllGather =================
            phb_cm = tc.tile_pool(name="phB", bufs=1)
            phb = phb_cm.__enter__()
            kt_full = phb.tile([P, CT, n], F32R, tag="ktf", name="kt_full")
            v_s = phb.tile([P, JT, dv], BF16, tag="vs", name="v_s")

            nc.gpsimd.collective_compute(
                "AllGather",
                mybir.AluOpType.bypass,
                replica_groups=groups,
                ins=[pack[:].opt()],
                outs=[pack_ag[:].opt()],
            )


            def assemble_rank(r):
                for ct in range(CT):
                    eng = nc.sync
                    eng.dma_start(
                        kt_full[:, ct, r * nsh:(r + 1) * nsh],
                        pack_ag[:][
                            r * PACK_ROWS + ct * P:
                            r * PACK_ROWS + (ct + 1) * P, :
                        ].bitcast(F32R),
                    )
                for it in range(IT):
                    base = r * PACK_ROWS + dqk + it * vph
                    eng = nc.sync if it % 2 == 0 else nc.scalar
                    eng.dma_start(
                        v_s[:, r * IT + it, :],
                        pack_ag[:][base:base + vph, :].bitcast(BF16).rearrange(
                            "ph (pl c) -> ph pl c", pl=vpl
                        ),
                    )

            # ================= Main attention loop =================
            # Streaming flash-attention: each 1024-wide chunk flows
            # MM -> row-max -> exp -> P^T transpose -> PV partial, with a
            # running-max rescale of the accumulator (no per-i-tile barrier).
            with (
                tc.tile_pool(name="mainA", bufs=8) as ma,
                tc.tile_pool(name="chunkp", bufs=2) as cp,
                tc.tile_pool(name="scores_psum", bufs=3, space="PSUM") as sps,
                tc.tile_pool(name="out_psum", bufs=2, space="PSUM") as ops,
            ):
                JPC = CHUNK // P  # j-tiles per chunk

                # flatten (i-tile, chunk) into one stream; PV lags scores by
                # one chunk so the PE never stalls on the exp->transpose tail
                NTOT = IT * NCH
                stats = []
                for it in range(IT):
                    st = {
                        "mneg": ma.tile([P, NCH], F32, tag="mneg", name="mneg"),
                        "sig": ma.tile([P, NCH], F32, tag="sig", name="sig"),
                        "nmr": ma.tile([P, NCH], F32, tag="nmr", name="nmr"),
                        "alpha": ma.tile([P, NCH], F32, tag="alpha", name="alpha"),
                        "asig": ma.tile([P, NCH], F32, tag="asig", name="asig"),
                        "rs": ma.tile([P, 1], F32, tag="rs", name="rs"),
                        "rinv": ma.tile([P, 1], F32, tag="rinv", name="rinv"),
                        "gam": ma.tile([P, NCH], F32, tag="gam", name="gam"),
                        "bet": ma.tile([P, NCH], F32, tag="bet", name="bet"),
                        "acc": ma.tile([P, dv], F32, tag="acc", name="acc"),
                    }
                    stats.append(st)

                import collections
                pend = collections.deque()  # (k, pt_c) with deep PV lag

                def do_scores(k):
                    ch, it = divmod(k, IT)
                    st = stats[it]
                    ps = sps.tile([P, CHUNK], F32, tag="s", name="ps")
                    for ct in range(CT):
                        for nn in range(NN):
                            nc.tensor.matmul(
                                ps[:, nn * 512:(nn + 1) * 512],
                                qt_s[:, ct, it * P:(it + 1) * P],
                                kt_full[
                                    :, ct,
                                    ch * CHUNK + nn * 512:
                                    ch * CHUNK + (nn + 1) * 512,
                                ],
                                start=(ct == 0),
                                stop=(ct == CT - 1),
                                skip_group_check=True,
                            )
                    nc.vector.reduce_max(
                        st["mneg"][:, ch:ch + 1], ps[:],
                        axis=mybir.AxisListType.X, negate=True,
                    )
                    if ch > 0:
                        nc.vector.tensor_tensor(
                            st["nmr"][:, ch:ch + 1], st["nmr"][:, ch - 1:ch],
                            st["mneg"][:, ch:ch + 1], op=mybir.AluOpType.min,
                        )
                    else:
                        nc.vector.tensor_copy(st["nmr"][:, :1], st["mneg"][:, :1])
                    p_c = cp.tile([P, CHUNK], BF16, tag="p", name="p_c", bufs=4)
                    nc.scalar.activation(
                        p_c[:], ps[:],
                        mybir.ActivationFunctionType.Exp,
                        bias=st["mneg"][:, ch:ch + 1],
                        scale=1.0,
                        accum_out=st["sig"][:, ch:ch + 1],
                    )
                    pt_c = cp.tile([P, JPC, P], BF16, tag="pt", name="pt_c", bufs=26)
                    nc.sync.dma_start_transpose(pt_c[:], p_c[:])
                    return pt_c

                def do_pv(k, pt_c):
                    ch, it = divmod(k, IT)
                    st = stats[it]
                    po = ops.tile([P, dv], F32, tag="po", name="po")
                    for j2 in range(JPC):
                        nc.tensor.matmul(
                            po[:], pt_c[:, j2, :],
                            v_s[:, ch * JPC + j2, :],
                            start=(j2 == 0), stop=(j2 == JPC - 1),
                        )
                    if ch == 0:
                        nc.vector.tensor_copy(st["acc"][:], po[:])
                    else:
                        nc.scalar.activation(
                            st["gam"][:, ch:ch + 1], st["nmr"][:, ch - 1:ch],
                            mybir.ActivationFunctionType.Exp,
                            bias=st["nmr"][:, ch:ch + 1], scale=-1.0,
                        )
                        nc.scalar.activation(
                            st["bet"][:, ch:ch + 1], st["mneg"][:, ch:ch + 1],
                            mybir.ActivationFunctionType.Exp,
                            bias=st["nmr"][:, ch:ch + 1], scale=-1.0,
                        )
                        nc.vector.tensor_scalar_mul(
                            st["acc"][:], st["acc"][:], st["gam"][:, ch:ch + 1]
                        )
                        nc.vector.scalar_tensor_tensor(
                            st["acc"][:], po[:], st["bet"][:, ch:ch + 1],
                            st["acc"][:],
                            op0=mybir.AluOpType.mult,
                            op1=mybir.AluOpType.add,
                        )
                    if ch == NCH - 1:
                        nc.scalar.activation(
                            st["alpha"][:], st["mneg"][:],
                            mybir.ActivationFunctionType.Exp,
                            bias=st["nmr"][:, NCH - 1:NCH], scale=-1.0,
                        )
                        nc.vector.tensor_mul(st["asig"][:], st["alpha"][:], st["sig"][:])
                        nc.vector.reduce_sum(
                            st["rs"][:], st["asig"][:], axis=mybir.AxisListType.X
                        )
                        nc.vector.reciprocal(st["rinv"][:], st["rs"][:])
                        nc.vector.tensor_scalar_mul(
                            st["acc"][:], st["acc"][:], st["rinv"][:]
                        )
                        nc.sync.dma_start(
                            out_ext.ap().rearrange("(it p) c -> p it c", p=P)[
                                :, it, :
                            ],
                            st["acc"][:],
                        )

                order = [g * IT + i2 for g in range(NCH) for i2 in range(IT)]
                LAG = min(24, max(1, len(order) - 1))
                for r in range(ncores):
                    assemble_rank(r)
                for k in order:
                    pt_now = do_scores(k)
                    pend.append((k, pt_now))
                    if len(pend) > LAG:
                        kk, pt = pend.popleft()
                        do_pv(kk, pt)
                while pend:
                    kk, pt = pend.popleft()
                    do_pv(kk, pt)

            phb_cm.__exit__(None, None, None)

    nc.finalize()
    return nc


_NC_CACHE = {}


def _get_nc(key):
    if key not in _NC_CACHE:
        n, d, dqk, dv, ncores = key
        _NC_CACHE[key] = build(n=n, d=d, dqk=dqk, dv=dv, ncores=ncores)
    return _NC_CACHE[key]


def run(x, Wq, Wk, Wv, trace=False):
    n, d = x.shape
    dqk = Wq.shape[0]
    dv = Wv.shape[0]
    ncores = N_CORES
    nsh = n // ncores
    nc = _get_nc((n, d, dqk, dv, ncores))

    x = np.ascontiguousarray(x, dtype=np.float32)
    Wq = np.ascontiguousarray(Wq, dtype=np.float32)
    Wk = np.ascontiguousarray(Wk, dtype=np.float32)
    Wv = np.ascontiguousarray(Wv, dtype=np.float32)

    in_maps = [
        {"x": x[r * nsh:(r + 1) * nsh], "Wq": Wq, "Wk": Wk, "Wv": Wv}
        for r in range(ncores)
    ]
    res = run_bass_kernel_spmd(
        nc, in_maps, core_ids=list(range(ncores)), trace=trace
    )
    out = np.concatenate([res.results[r]["out"] for r in range(ncores)], axis=0)
    return out, res


def kernel(x, Wq, Wk, Wv):
    out, _ = run(x, Wq, Wk, Wv)
    return out
